# revision 26
# baseline (speedup 1.0000x reference)
"""Trainium2 Bass kernel for the multi-scale detection loss.

Strategy: every term of the loss is masked by pos_mask, so only pred values at
the <=60 target cells per (batch, scale) matter.  Host computes the target
cell indices / collision-winner masks / multi-hot class targets from the tiny
targets tensors, lays the predictions out channel-last (padded to 16 f32 per
cell) and shards the batch across 8 cores.  The device kernel:
  1. dma_gathers the 256B records covering each winner cell from the pred
     tables resident in HBM (3 calls, ~1.5k descriptors),
  2. extracts each cell's 16-float record via a select mask,
  3. computes BCE + IoU + inner-IoU terms on the gathered slots (the full and
     inner IoU pipelines run fused on f/i-stacked tensors),
  4. reduces to 12 partial sums (4 quantities x 3 scales),
  5. AllReduces across the 8 cores,
  6. applies the final normalization/weighting and writes the 3-vector.
"""
import numpy as np

import concourse.bacc as bacc
import concourse.bass as bass
import concourse.tile as tile
import concourse.mybir as mybir
from concourse.bass_utils import run_bass_kernel_spmd

F32 = mybir.dt.float32
I16 = mybir.dt.int16
ALU = mybir.AluOpType
ACT = mybir.ActivationFunctionType

B, T, NCLS = 64, 60, 6
NCORES = 8
BLOC = B // NCORES            # 8 batches per core
SCALES = [(160, 160), (80, 80), (40, 40)]
CH = 11
REC = 16                      # padded record size (f32) per cell
NJ = 12                       # slot columns: j 0-1 p3a, 2-3 p3b, 4-7 p4, 8-11 p5
ROWS_3 = 4 * 160 * 160 * REC // 64     # 25600 rows per half of p3
ROWS_45 = (BLOC * 80 * 80 + BLOC * 40 * 40) * REC // 64   # 16000
N45_P4 = BLOC * 80 * 80                # p4 cell count inside tab45
# meta layout per slot: sel(64) | mh6(6) | tbox(4) | wmask6(6) | wmask(1)
NMETA = 64 + 6 + 4 + 6 + 1


# ---------------------------------------------------------------- host prep
def _host_prep(targets_cls, targets_box):
    """Per scale: winner list per batch. Winner = LAST occurrence of a
    duplicated cell (XLA scatter .set semantics); multi-hot = union of classes
    of all boxes mapping to that cell."""
    out = []
    tc = np.asarray(targets_cls)
    for (H, W) in SCALES:
        x = targets_box[..., 0].astype(np.float32)
        y = targets_box[..., 1].astype(np.float32)
        gx = np.clip((x * np.float32(W)).astype(np.int32), 0, W - 1)
        gy = np.clip((y * np.float32(H)).astype(np.int32), 0, H - 1)
        cell = gy.astype(np.int64) * W + gx
        winners = []
        for b in range(B):
            groups = {}
            for t in range(T):
                groups.setdefault(int(cell[b, t]), []).append(t)
            lst = []
            for c, ts in groups.items():
                mh = np.zeros(NCLS, np.float32)
                for t in ts:
                    mh[tc[b, t]] = 1.0
                lst.append((c, ts[-1], mh))
            winners.append(lst)
        out.append(winners)
    return out


def _wrap_idx16(idx, ncols):
    """idx list -> [128, ncols] int16 tile (16-partition wrap, replicated x8)."""
    n = ncols * 16
    buf = np.zeros(n, np.int16)
    buf[:len(idx)] = idx
    w = buf.reshape(ncols, 16).T           # [16, ncols], idx k at [k%16, k//16]
    return np.tile(w, (8, 1)).astype(np.int16)


def _build_core_inputs(pred_p3, pred_p4, pred_p5, targets_cls, targets_box):
    prep = _host_prep(targets_cls, targets_box)
    tbox_np = np.asarray(targets_box, dtype=np.float32)

    in_maps = []
    for core in range(NCORES):
        b0 = core * BLOC

        def mk_table(parts):
            recs = []
            for p, lo, hi in parts:
                cl = np.moveaxis(np.asarray(p[lo:hi], np.float32), 1, -1)
                cells = cl.reshape(-1, CH)
                pad = np.zeros((cells.shape[0], REC), np.float32)
                pad[:, :CH] = cells
                recs.append(pad)
            return np.concatenate(recs).reshape(-1, 64)

        tab3a = mk_table([(pred_p3, b0, b0 + 4)])
        tab3b = mk_table([(pred_p3, b0 + 4, b0 + 8)])
        tab45 = mk_table([(pred_p4, b0, b0 + 8), (pred_p5, b0, b0 + 8)])

        meta = np.zeros((128, NJ, NMETA), np.float32)
        idx_lists = {"idx3a": [], "idx3b": [], "idx45": []}

        regions = [
            (0, range(0, 4), 0, "idx3a", lambda bl: bl * 160 * 160),
            (0, range(4, 8), 2, "idx3b", lambda bl: (bl - 4) * 160 * 160),
            (1, range(0, 8), 4, "idx45", lambda bl: bl * 80 * 80),
            (2, range(0, 8), 8, "idx45", lambda bl: N45_P4 + bl * 40 * 40),
        ]
        for si, bls, j0, key, cell_off in regions:
            if si == 2:      # p5 slots start at fixed offset 512 in idx45
                idx_lists[key].extend([0] * (512 - len(idx_lists[key])))
            k = 0
            for bl in bls:
                b = b0 + bl
                for c, t_w, mh in prep[si][b]:
                    g = cell_off(bl) + c
                    p, j = k % 128, j0 + k // 128
                    idx_lists[key].append(g // 4)
                    v = g % 4
                    meta[p, j, v * 16:(v + 1) * 16] = 1.0        # sel
                    meta[p, j, 64:70] = mh
                    meta[p, j, 70:74] = tbox_np[b, t_w]
                    meta[p, j, 74:80] = 1.0                      # wmask6
                    meta[p, j, 80] = 1.0                         # wmask
                    k += 1
            cap = {"idx3a": 256, "idx3b": 256}.get(key)
            if cap is not None:
                idx_lists[key].extend([0] * (cap - len(idx_lists[key])))
        idx_lists["idx45"].extend([0] * (1024 - len(idx_lists["idx45"])))

        idxs = np.concatenate([
            _wrap_idx16(idx_lists["idx3a"], 16),
            _wrap_idx16(idx_lists["idx3b"], 16),
            _wrap_idx16(idx_lists["idx45"], 64),
        ], axis=1)                                               # [128, 96]
        in_maps.append(dict(tab3a=tab3a, tab3b=tab3b, tab45=tab45,
                            idxs=idxs, meta=meta))
    return in_maps


# ------------------------------------------------------------- bass program
def build_program(debug_outs=False, single_core=False):
    """single_core=True replaces the AllReduce with a local copy — used only
    for cost-model timeline estimation (TimelineSim is single-core-only)."""
    nc = bacc.Bacc("TRN2", target_bir_lowering=False, debug=False,
                   num_devices=1 if single_core else NCORES)
    tab3a = nc.dram_tensor("tab3a", [ROWS_3, 64], F32, kind="ExternalInput")
    tab3b = nc.dram_tensor("tab3b", [ROWS_3, 64], F32, kind="ExternalInput")
    tab45 = nc.dram_tensor("tab45", [ROWS_45, 64], F32, kind="ExternalInput")
    idxs = nc.dram_tensor("idxs", [128, 96], I16, kind="ExternalInput")
    meta = nc.dram_tensor("meta", [128, NJ, NMETA], F32, kind="ExternalInput")
    out12 = nc.dram_tensor("out12", [12, 1], F32, kind="ExternalOutput")
    if debug_outs:
        dbg_G = nc.dram_tensor("dbg_G", [128, NJ, REC], F32, kind="ExternalOutput")
        dbg_partials = nc.dram_tensor("dbg_partials", [128, 12], F32,
                                      kind="ExternalOutput")

    with tile.TileContext(nc) as tc:
        with (
            tc.tile_pool(name="sb", bufs=1) as sb,
            tc.tile_pool(name="pp", bufs=1, space="PSUM") as pp,
            tc.tile_pool(name="dp", bufs=1, space="DRAM") as dp,
        ):
            idxs_sb = sb.tile([128, 96], I16)
            meta_sb = sb.tile([128, NJ, NMETA], F32)
            nc.sync.dma_start(idxs_sb[:], idxs[:])
            nc.sync.dma_start(meta_sb[:], meta[:])
            sel = meta_sb[:, :, 0:64]
            mh6 = meta_sb[:, :, 64:70]
            tboxm = meta_sb[:, :, 70:74]
            wmask6 = meta_sb[:, :, 74:80]
            wmask2 = meta_sb[:, :, 74:76]
            wmask = meta_sb[:, :, 80:81]

            G2 = sb.tile([128, NJ, 64], F32)
            # big gather first: its SDMA flight overlaps the small emissions
            nc.gpsimd.dma_gather(G2[:, 4:12, :], tab45[:], idxs_sb[:, 32:96],
                                 1024, 1024, 64)
            nc.gpsimd.dma_gather(G2[:, 0:2, :], tab3a[:], idxs_sb[:, 0:16],
                                 256, 256, 64)
            nc.gpsimd.dma_gather(G2[:, 2:4, :], tab3b[:], idxs_sb[:, 16:32],
                                 256, 256, 64)

            vec = nc.vector

            # extract each slot's 16-float record: G = sum of 4 masked chunks
            Gm = sb.tile([128, NJ, 64], F32)
            vec.tensor_tensor(Gm[:], G2[:], sel, op=ALU.mult)
            ha = sb.tile([128, NJ, 32], F32)
            vec.tensor_tensor(ha[:], Gm[:, :, 0:32], Gm[:, :, 32:64], op=ALU.add)
            G = sb.tile([128, NJ, REC], F32)
            vec.tensor_tensor(G[:], ha[:, :, 0:16], ha[:, :, 16:32], op=ALU.add)

            L = G[:, :, 0:6]

            # BCE: (max(L,0) - L*mh + log1p(exp(-|L|))) * w   (ACT for abs/
            # exp/ln/relu — all four live in one activation table)
            aabs = sb.tile([128, NJ, NCLS], F32)
            nc.scalar.activation(aabs[:], L, ACT.Abs)
            ex = sb.tile([128, NJ, NCLS], F32)
            nc.scalar.activation(ex[:], aabs[:], ACT.Exp, scale=-1.0)
            lg = sb.tile([128, NJ, NCLS], F32)
            nc.scalar.activation(lg[:], ex[:], ACT.Ln, bias=1.0)
            rl = sb.tile([128, NJ, NCLS], F32)
            nc.scalar.activation(rl[:], L, ACT.Relu)
            pm = sb.tile([128, NJ, NCLS], F32)
            vec.tensor_tensor(pm[:], L, mh6, op=ALU.mult)
            rp = sb.tile([128, NJ, NCLS], F32)
            vec.tensor_tensor(rp[:], rl[:], pm[:], op=ALU.subtract)
            bce = sb.tile([128, NJ, NCLS], F32)
            vec.tensor_tensor(bce[:], rp[:], lg[:], op=ALU.add)
            bcew = sb.tile([128, NJ, NCLS], F32)
            vec.tensor_tensor(bcew[:], bce[:], wmask6, op=ALU.mult)

            # masked box pred (zeroes padding slots so IoU stays finite)
            Pm = sb.tile([128, NJ, 4], F32)
            vec.tensor_tensor(Pm[:], G[:, :, 7:11], wmask6[:, :, 0:4],
                              op=ALU.mult)
            Pxy, Pwh = Pm[:, :, 0:2], Pm[:, :, 2:4]
            Txy, Twh = tboxm[:, :, 0:2], tboxm[:, :, 2:4]

            # fused full+inner IoU: last dim stacks (full_x, full_y, in_x, in_y)
            Pxy2 = sb.tile([128, NJ, 4], F32)
            vec.tensor_copy(Pxy2[:, :, 0:2], Pxy)
            vec.tensor_copy(Pxy2[:, :, 2:4], Pxy)
            Txy2 = sb.tile([128, NJ, 4], F32)
            vec.tensor_copy(Txy2[:, :, 0:2], Txy)
            vec.tensor_copy(Txy2[:, :, 2:4], Txy)
            ph = sb.tile([128, NJ, 4], F32)          # half extents
            vec.tensor_scalar_mul(ph[:, :, 0:2], Pwh, 0.5)
            vec.tensor_scalar_mul(ph[:, :, 2:4], Pwh, float(np.float32(0.7) * np.float32(0.5)))
            th = sb.tile([128, NJ, 4], F32)
            vec.tensor_scalar_mul(th[:, :, 0:2], Twh, 0.5)
            vec.tensor_scalar_mul(th[:, :, 2:4], Twh, float(np.float32(0.7) * np.float32(0.5)))

            P1 = sb.tile([128, NJ, 4], F32)
            vec.tensor_tensor(P1[:], Pxy2[:], ph[:], op=ALU.subtract)
            P2 = sb.tile([128, NJ, 4], F32)
            vec.tensor_tensor(P2[:], Pxy2[:], ph[:], op=ALU.add)
            T1 = sb.tile([128, NJ, 4], F32)
            vec.tensor_tensor(T1[:], Txy2[:], th[:], op=ALU.subtract)
            T2 = sb.tile([128, NJ, 4], F32)
            vec.tensor_tensor(T2[:], Txy2[:], th[:], op=ALU.add)
            lo = sb.tile([128, NJ, 4], F32)
            vec.tensor_tensor(lo[:], P1[:], T1[:], op=ALU.max)
            hi = sb.tile([128, NJ, 4], F32)
            vec.tensor_tensor(hi[:], P2[:], T2[:], op=ALU.min)
            d = sb.tile([128, NJ, 4], F32)
            vec.tensor_tensor(d[:], hi[:], lo[:], op=ALU.subtract)
            dr = sb.tile([128, NJ, 4], F32)
            vec.tensor_scalar_max(dr[:], d[:], 0.0)
            wp = sb.tile([128, NJ, 4], F32)
            vec.tensor_tensor(wp[:], P2[:], P1[:], op=ALU.subtract)
            wt = sb.tile([128, NJ, 4], F32)
            vec.tensor_tensor(wt[:], T2[:], T1[:], op=ALU.subtract)

            # pairwise x*y products -> (full, inner) per slot
            inter = sb.tile([128, NJ, 2], F32)
            vec.tensor_tensor(inter[:], dr[:, :, 0:4:2], dr[:, :, 1:4:2],
                              op=ALU.mult)
            a1 = sb.tile([128, NJ, 2], F32)
            vec.tensor_tensor(a1[:], wp[:, :, 0:4:2], wp[:, :, 1:4:2],
                              op=ALU.mult)
            a2 = sb.tile([128, NJ, 2], F32)
            vec.tensor_tensor(a2[:], wt[:, :, 0:4:2], wt[:, :, 1:4:2],
                              op=ALU.mult)
            u = sb.tile([128, NJ, 2], F32)
            vec.tensor_tensor(u[:], a1[:], a2[:], op=ALU.add)
            u2 = sb.tile([128, NJ, 2], F32)
            vec.tensor_tensor(u2[:], u[:], inter[:], op=ALU.subtract)
            union = sb.tile([128, NJ, 2], F32)
            vec.tensor_scalar_add(union[:], u2[:], 1e-7)
            urec = sb.tile([128, NJ, 2], F32)
            vec.reciprocal(urec[:], union[:])
            iou = sb.tile([128, NJ, 2], F32)
            vec.tensor_tensor(iou[:], inter[:], urec[:], op=ALU.mult)
            iw = sb.tile([128, NJ, 2], F32)
            vec.tensor_tensor(iw[:], iou[:], wmask2, op=ALU.mult)
            term = sb.tile([128, NJ, 2], F32)
            vec.tensor_tensor(term[:], wmask2, iw[:], op=ALU.subtract)

            # partial sums: columns = [cls x3, iou x3, inner x3, npos x3]
            partials = sb.tile([128, 12], F32)
            bcs = sb.tile([128, NJ], F32)
            vec.tensor_reduce(bcs[:], bcew[:], axis=mybir.AxisListType.X,
                              op=ALU.add)
            vec.tensor_reduce(partials[:, 0:3],
                              bcs[:].rearrange("p (s j) -> p s j", s=3),
                              axis=mybir.AxisListType.X, op=ALU.add)
            vec.tensor_reduce(partials[:, 3:6],
                              term[:, :, 0:1].rearrange("p (s j) o -> p s (j o)", s=3),
                              axis=mybir.AxisListType.X, op=ALU.add)
            vec.tensor_reduce(partials[:, 6:9],
                              term[:, :, 1:2].rearrange("p (s j) o -> p s (j o)", s=3),
                              axis=mybir.AxisListType.X, op=ALU.add)
            vec.tensor_reduce(partials[:, 9:12],
                              wmask.rearrange("p (s j) o -> p s (j o)", s=3),
                              axis=mybir.AxisListType.X, op=ALU.add)

            # partition reduction via matmul with a ones vector
            ones = sb.tile([128, 1], F32)
            vec.memset(ones[:], 1.0)
            pv = pp.tile([12, 1], F32)
            nc.tensor.matmul(pv[:], lhsT=partials[:], rhs=ones[:],
                             start=True, stop=True)
            av = sb.tile([12, 1], F32)
            vec.tensor_copy(av[:], pv[:])

            cc_in = dp.tile([12, 1], F32)
            cc_out = dp.tile([12, 1], F32)
            nc.sync.dma_start(cc_in[:], av[:])
            if single_core:
                nc.sync.dma_start(cc_out[:], cc_in[:])
            else:
                nc.gpsimd.collective_compute(
                    "AllReduce", ALU.add,
                    replica_groups=[list(range(NCORES))],
                    ins=[cc_in.opt()], outs=[cc_out.opt()],
                )
            nc.sync.dma_start(out12[:], cc_out[:])
            if debug_outs:
                nc.sync.dma_start(dbg_G[:], G[:])
                nc.sync.dma_start(dbg_partials[:], partials[:])

    # Force all ACT funcs onto one table (natural_log_exp_and_others holds
    # Abs/Exp/Ln/Relu) so only one LoadActFuncSet is emitted. Table ids are
    # positional, so empty the others instead of filtering.
    import concourse.hw_specs as hw_specs
    orig = bacc.get_activation_tables
    keep = "natural_log_exp_and_others"

    def patched(arch):
        t = orig(arch)
        return {k: (v if k == keep else set()) for k, v in t.items()}

    bacc.get_activation_tables = patched
    try:
        nc.compile()
    finally:
        bacc.get_activation_tables = orig
    return nc


_NC_CACHE = []


def _run(in_maps, **kw):
    if not _NC_CACHE:
        _NC_CACHE.append(build_program())
    return run_bass_kernel_spmd(_NC_CACHE[0], in_maps, list(range(NCORES)), **kw)


def _final_combine(p12):
    """Unshard step: exact f32 replication of the reference's final
    normalization, applied to the device-AllReduced component sums."""
    f = np.float32
    p = np.asarray(p12, np.float32)
    npos = (p[9:12] + f(1e-8)).astype(np.float32)
    cls_t = (p[0:3] / npos).astype(np.float32)
    iou_t = (p[3:6] / npos).astype(np.float32)
    inn_t = (p[6:9] / npos).astype(np.float32)
    cls_total = f(0.0)
    box_total = f(0.0)
    for s in range(3):
        inner_loss = f(0.5) * iou_t[s] + f(0.5) * inn_t[s]
        box_loss = f(0.5) * iou_t[s] + f(0.5) * inner_loss
        cls_total = cls_total + cls_t[s]
        box_total = box_total + box_loss
    cls_total = cls_total / f(3.0)
    box_total = box_total / f(3.0)
    total = f(0.5) * cls_total + f(7.5) * box_total
    return np.array([total, cls_total, box_total], np.float32)


def kernel(pred_p3, pred_p4, pred_p5, targets_cls, targets_box):
    in_maps = _build_core_inputs(pred_p3, pred_p4, pred_p5,
                                 targets_cls, targets_box)
    res = _run(in_maps)
    return _final_combine(np.asarray(res.results[0]["out12"]).reshape(12))


def kernel_profiled(pred_p3, pred_p4, pred_p5, targets_cls, targets_box):
    """Same as kernel() but returns (out, exec_time_ns) when profiling works."""
    in_maps = _build_core_inputs(pred_p3, pred_p4, pred_p5,
                                 targets_cls, targets_box)
    res = _run(in_maps, trace=True)
    out = _final_combine(np.asarray(res.results[0]["out12"]).reshape(12))
    return out, res.exec_time_ns
